# revision 30
# baseline (speedup 1.0000x reference)
"""Multi-head graph attention kernel for Trainium2 (8 NeuronCores).

Problem: B=8, N=1024, F_IN=F_OUT=128, H=8, D_K=16, sparse 0/1 adjacency mask.
Sharding: data-parallel over B - core b processes batch element b.

Math (identical to reference up to fp rounding):
    Q = X@Wq.T + bq ; K = X@Wk.T + bk ; V0 = X@Wv.T
    S = Q_h @ K_h.T ;  P = exp(S/4) * A01   (multiplicative 0/1 mask after
    exp == masked_fill(-1e9)+softmax numerator; softmax is shift-invariant
    and |S/4| is small so no max-subtract is needed)
    out = (P @ V0) / rowsum(P) @ Wo.T + (bo + Wo@bv)   (bv folds into bias
    because attn rows sum to 1)

Device layout (everything transposed on host so the device never transposes):
    xt   [128c, 1024n]      = X.T                   (bf16)
    at01 [128p, 8j, 1024q]  = A.T blocked over k    (bf16, 0/1)
    Heads are processed in two groups g in {0,1} of 4 heads; head slot a in
    {0..3} lives at partition base 32a with rows 16..31 zero-padded so the
    D_K=16 contraction can use 4-way tile_position row tiling on the PE.
    Scores are computed transposed: ST[k, q], one PSUM tile per
    (g, j-block, head-slot) covering the full q=1024 so the exp runs 1024
    wide and the adjacency multiply is a single contiguous DVE op at 2x
    bf16 rate. P.T streams straight into the P@V matmul, with a ones
    column appended to V giving rowsum for free; rowsum reciprocals are
    broadcast across each 32-partition quadrant with STREAM_SHUFFLE.
"""

import sys

sys.path.insert(0, "/opt/trn_rl_repo")

import numpy as np
import ml_dtypes

BF16 = ml_dtypes.bfloat16

B, N, C, F, H, D = 8, 1024, 128, 128, 8, 16
NB = N // 128  # 8 k-blocks

_CACHED = None


def _split_multi_waits(nc):
    """This toolchain's walrus accepts at most ONE sync wait per instruction.
    Tile emits several; split the extras onto preceding same-engine NOPs."""
    import concourse.mybir as mybir

    for f in nc.m.functions:
        for blk in f.blocks:
            new = []
            for inst in blk.instructions:
                si = inst.sync_info
                if si is not None and si.on_wait is not None and len(si.on_wait) > 1:
                    waits = list(si.on_wait)
                    for w in waits[:-1]:
                        nop = mybir.InstNoOp(
                            name=nc.get_next_instruction_name(), ins=[], outs=[])
                        nop.engine = inst.engine
                        nop.sync_info = mybir.SyncInfo(on_wait=[w], on_update=[])
                        new.append(nop)
                    inst.sync_info = mybir.SyncInfo(
                        on_wait=[waits[-1]], on_update=list(si.on_update or []))
                new.append(inst)
            del blk.instructions[:]
            for i in new:
                blk.instructions.append(i)


def _build_nc(repeat=1):
    import concourse.bass as bass
    import concourse.tile as tile
    from concourse import mybir

    f32 = mybir.dt.float32
    bf16 = mybir.dt.bfloat16
    AF = mybir.ActivationFunctionType

    nc = bass.Bass()

    xt_d = nc.declare_dram_parameter("xt", [C, N], bf16, isOutput=False)
    at_d = nc.declare_dram_parameter("at01", [128, NB, N], bf16, isOutput=False)
    wblob_d = nc.declare_dram_parameter("wblob", [128, 7, 128], bf16, isOutput=False)
    bblob_d = nc.declare_dram_parameter("bblob", [128, 5], f32, isOutput=False)
    yt_d = nc.declare_dram_parameter("yt", [F, N], f32, isOutput=True)

    with tile.TileContext(nc) as tc:
        with (
            tc.tile_pool(name="consts", bufs=1) as cp,
            tc.tile_pool(name="sbwork", bufs=2) as wp,
            tc.tile_pool(name="ptp", bufs=10) as ptp,
            tc.tile_pool(name="smalls", bufs=2) as smp,
            tc.tile_pool(name="ps_big", bufs=2, space="PSUM") as pbig,
            tc.tile_pool(name="ps_ovy", bufs=1, space="PSUM") as povy,
            tc.tile_pool(name="dramp", bufs=2, space="DRAM") as dp,
        ):
            # ---- consts + input DMAs (outside the repeat loop) ----
            scratch = cp.tile([1, 8], f32, name="scratch")
            nc.vector.memset(scratch[:], 0.0)
            scratch2 = cp.tile([1, 8], f32, name="scratch2")
            # dummy exp: pulls the ACT table load to t=0 so it overlaps DMAs
            nc.scalar.activation(out=scratch2[:], in_=scratch[:], func=AF.Exp)

            wblob = cp.tile([128, 7, 128], bf16, name="wblob")
            nc.sync.dma_start(out=wblob[:], in_=wblob_d[:, :, :])
            xt_sb = cp.tile([C, N], bf16, name="xt_sb")
            nc.sync.dma_start(out=xt_sb[:], in_=xt_d[:, :])
            bblob = cp.tile([128, 5], f32, name="bblob")
            nc.sync.dma_start(out=bblob[:], in_=bblob_d[:, :])
            at_sb = cp.tile([128, NB, N], bf16, name="at_sb")
            nc.sync.dma_start(out=at_sb[:, 0:2, :], in_=at_d[:, 0:2, :])
            nc.sync.dma_start(out=at_sb[:, 2:NB, :], in_=at_d[:, 2:NB, :])

            wq_sb = [wblob[:, g, :] for g in range(2)]
            wk_sb = [wblob[:, 2 + g, :] for g in range(2)]
            wv_sb = wblob[:, 4, :]
            wo_sb = [wblob[:, 5 + g, :] for g in range(2)]
            bq_sb = [bblob[:, g : g + 1] for g in range(2)]
            bk_sb = [bblob[:, 2 + g : 3 + g] for g in range(2)]
            bfin_sb = bblob[:, 4:5]

            def body(deferred, prev=None):
                """One iteration. With deferred=True, the previous
                iteration's g1-normalization and output projection are
                finished here, interleaved into this iteration's g0
                attention. `prev` carries the previous body's accumulator
                handles when both sit in the same basic block (unrolled);
                across the hardware-loop back edge prev is None and the
                same work reads this emission's own tiles, whose addresses
                the ring reuse makes equal to the previous iteration's.
                emit_tail() finishes the last iteration."""
                qt_sb = [wp.tile([128, N], bf16, tag=f"qt{g}", name=f"qt{g}")
                         for g in range(2)]
                kt_sb = [wp.tile([128, N], bf16, tag=f"kt{g}", name=f"kt{g}")
                         for g in range(2)]
                vaug = wp.tile([128, NB, H, D + 1], bf16, tag="vaug",
                               name="vaug")
                otn = [wp.tile([128, N], bf16, tag=f"otn{g}", name=f"otn{g}")
                       for g in range(2)]
                yt_sb = wp.tile([F, N], f32, tag="yt_sb", name="yt_sb")
                ov2 = [None, None]
                if deferred and prev is None:
                    # first body of a block: allocate up front; the deferred
                    # reads below then target this emission's tiles, whose
                    # ring addresses alias the previous block's last body
                    ov2[0] = povy.tile([128, N], f32, tag="ovy0", name="ov0")
                    ov2[1] = povy.tile([128, N], f32, tag="ovy1", name="ov1")
                    prev = {"ov2": ov2, "otn": otn, "yt": yt_sb}

                def qk_chunk(g):
                    qps = pbig.tile([128, N], f32, tag="big", name="qps")
                    nc.tensor.matmul(qps[:, 0:512], lhsT=wq_sb[g],
                                     rhs=xt_sb[:, 0:512])
                    nc.tensor.matmul(qps[:, 512:N], lhsT=wq_sb[g],
                                     rhs=xt_sb[:, 512:N])
                    nc.vector.tensor_scalar_add(qt_sb[g][:], qps[:], bq_sb[g])
                    kps = pbig.tile([128, N], f32, tag="big", name="kps")
                    nc.tensor.matmul(kps[:, 0:512], lhsT=wk_sb[g],
                                     rhs=xt_sb[:, 0:512])
                    nc.tensor.matmul(kps[:, 512:N], lhsT=wk_sb[g],
                                     rhs=xt_sb[:, 512:N])
                    nc.vector.tensor_scalar_add(kt_sb[g][:], kps[:], bk_sb[g])

                vt_box = {}

                def vproj_chunk(half):
                    if half == 0:
                        vt_box["vt"] = pbig.tile([128, N], f32, tag="big",
                                                 name="vps")
                    vt = vt_box["vt"]
                    for j in range(4 * half, 4 * half + 4):
                        nc.tensor.matmul(vt[:, j * 128 : (j + 1) * 128],
                                         lhsT=xt_sb[:, j * 128 : (j + 1) * 128],
                                         rhs=wv_sb)

                def vcopy_chunk(half):
                    if half == 0:
                        nc.vector.memset(vaug[:, :, :, D : D + 1], 1.0)
                    vt = vt_box["vt"]
                    for j in range(4 * half, 4 * half + 4):
                        nc.vector.tensor_copy(
                            out=vaug[:, j, :, 0:D],
                            in_=vt[:, j * 128 : (j + 1) * 128].rearrange(
                                "p (h d) -> p h d", d=D))

                def norm_stages(g, n_ov2=None, n_otn=None):
                    # rowsums sit at partition 32a+16 of ov2[g]. Compact the
                    # 4 scattered rows to [128,32] via a DRAM round-trip (DMA
                    # handles strided/broadcast APs engines cannot), run the
                    # exact DVE reciprocal on 32 columns, broadcast each
                    # slot's reciprocal row over its 16 head partitions on
                    # the way back. Staged so the DMA latencies overlap the
                    # surrounding attention work.
                    ovs = smp.tile([128, N], f32, tag="ovs", name="ovs")
                    rsc = smp.tile([128, 32], f32, tag="rsc", name="rsc")
                    rsv = smp.tile([128, 32], f32, tag="rsv", name="rsv")
                    rsq = smp.tile([128, N], f32, tag="rsq", name="rsq")
                    rsd = dp.tile([4, N], f32, tag="rsd", name="rsd")
                    rsd2 = dp.tile([4, N], f32, tag="rsd2", name="rsd2")
                    if n_ov2 is None:
                        n_ov2, n_otn = ov2[g], otn[g]

                    def s1():
                        nc.vector.tensor_copy(out=ovs[:], in_=n_ov2[:])
                        for a in range(4):
                            nc.sync.dma_start(
                                out=rsd[a : a + 1, :],
                                in_=ovs[32 * a + D : 32 * a + D + 1, :])
                        d0 = rsd[0:1, 0:1]
                        nc.sync.dma_start(
                            out=rsc[:],
                            in_=bass.AP(tensor=d0.tensor, offset=d0.offset,
                                        ap=[[32, 128], [1, 32]]))

                    def s2():
                        nc.vector.reciprocal(out=rsv[:], in_=rsc[:])
                        d2 = rsd2[0:1, 0:1]
                        nc.sync.dma_start(
                            out=bass.AP(tensor=d2.tensor, offset=d2.offset,
                                        ap=[[32, 128], [1, 32]]),
                            in_=rsv[:])
                        for a in range(4):
                            # broadcast over the full 32-row quadrant (the
                            # 16 pad rows too) so no stale SBUF can leak
                            # inf/NaN into otn's pad rows
                            row = rsd2[a : a + 1, :]
                            nc.sync.dma_start(
                                out=rsq[32 * a : 32 * a + 32, :],
                                in_=bass.AP(tensor=row.tensor,
                                            offset=row.offset,
                                            ap=[[0, 32], [1, N]]))

                    def s3():
                        nc.vector.tensor_mul(n_otn[:], n_ov2[:], rsq[:])

                    return [s1, s2, s3]

                def phase3_chunk(qh, p_otn=None, p_yt=None):
                    if p_otn is None:
                        p_otn, p_yt = otn, yt_sb
                    q0 = qh * 512
                    yps = pbig.tile([128, N], f32, tag="big", name="yps")
                    nc.tensor.matmul(yps[:, 0:512], lhsT=wo_sb[0],
                                     rhs=p_otn[0][:, q0 : q0 + 512],
                                     start=True, stop=False)
                    nc.tensor.matmul(yps[:, 0:512], lhsT=wo_sb[1],
                                     rhs=p_otn[1][:, q0 : q0 + 512],
                                     start=False, stop=True)
                    nc.vector.tensor_scalar_add(p_yt[:, q0 : q0 + 512],
                                                yps[:, 0:512], bfin_sb)
                    nc.sync.dma_start(out=yt_d[:, q0 : q0 + 512],
                                      in_=p_yt[:, q0 : q0 + 512])

                def score_mm(g, j, a, qh, sps):
                    nc.tensor.matmul(
                        sps[:, qh * 512 : (qh + 1) * 512],
                        lhsT=kt_sb[g][32 * a : 32 * a + 32,
                                      j * 128 : (j + 1) * 128],
                        rhs=qt_sb[g][32 * a : 32 * a + 32,
                                     qh * 512 : (qh + 1) * 512],
                        start=True, stop=True,
                        skip_group_check=True,
                        tile_position=(32 * a, 0),
                    )

                def exp_mask(j, sps):
                    pt = ptp.tile([128, N], bf16, tag="pt", name="pt")
                    nc.scalar.activation(out=pt[:], in_=sps[:],
                                         func=AF.Exp, scale=0.25)
                    nc.vector.tensor_mul(pt[:], pt[:], at_sb[:, j, :])
                    return pt

                def pv_block(g, j, pts):
                    # qh-major: consecutive matmuls hit different 32-col
                    # strips of the array and stream concurrently
                    for qh in range(2):
                        for a in range(4):
                            h = 4 * g + a
                            nc.tensor.matmul(
                                ov2[g][32 * a : 32 * a + D + 1,
                                       qh * 512 : (qh + 1) * 512],
                                lhsT=vaug[:, j, h, :],
                                rhs=pts[a][:, qh * 512 : (qh + 1) * 512],
                                start=(j == 0),
                                stop=(j == NB - 1),
                                skip_group_check=True,
                                tile_position=(0, 32 * a),
                            )

                # body emission: Q/K of g0 first so the first scores (and
                # the ACT pipeline) restart with minimal latency at the
                # iteration boundary; everything else trickles in as
                # pending chunks popped between attention steps.
                qk_chunk(0)
                pending = [lambda: vproj_chunk(0), lambda: vcopy_chunk(0),
                           lambda: vproj_chunk(1), lambda: vcopy_chunk(1),
                           lambda: qk_chunk(1)]
                if deferred:
                    # finish the previous body: its g1 normalization and
                    # output projection, interleaved into our g0 attention
                    pending += norm_stages(1, prev["ov2"][1],
                                           prev["otn"][1])
                    pending += [lambda: phase3_chunk(0, prev["otn"],
                                                     prev["yt"]),
                                lambda: phase3_chunk(1, prev["otn"],
                                                     prev["yt"])]

                pvprev = None
                for g in range(2):
                    if ov2[g] is None:
                        ov2[g] = povy.tile([128, N], f32, tag=f"ovy{g}",
                                           name=f"ov{g}")
                    for j in range(NB):
                        pair01 = [pbig.tile([128, N], f32, tag="big",
                                            name="sps") for _ in range(2)]
                        for qh in range(2):
                            for si in range(2):
                                score_mm(g, j, si, qh, pair01[si])
                        if pvprev is not None:
                            pv_block(*pvprev)
                        if pending:
                            pending.pop(0)()
                        if j == 0 and pending:
                            # vcopy must land before this j's PV block
                            pending.pop(0)()
                        pair23 = [pbig.tile([128, N], f32, tag="big",
                                            name="sps") for _ in range(2)]
                        for qh in range(2):
                            for si in range(2):
                                score_mm(g, j, 2 + si, qh, pair23[si])
                        pts = [exp_mask(j, s) for s in pair01 + pair23]
                        pvprev = (g, j, pts)
                    pv_block(*pvprev)
                    pvprev = None
                    if g == 0:
                        pending += norm_stages(0)
                for p in pending:
                    p()

                if not deferred:
                    for s in norm_stages(1):
                        s()
                    phase3_chunk(0)
                    phase3_chunk(1)
                return {"ov2": ov2, "otn": otn, "yt": yt_sb,
                        "norm_stages": norm_stages,
                        "phase3_chunk": phase3_chunk}

            def emit_tail(h):
                for s in h["norm_stages"](1):
                    s()
                h["phase3_chunk"](0)
                h["phase3_chunk"](1)

            def block(nbodies):
                # group of iterations: body k+1 finishes body k's tail work
                # interleaved into its own attention; the LAST body's
                # accumulators are finished by the NEXT block's first body
                # through ring-address aliasing (the For_i barrier orders
                # it). The tiny reads below retire those written-but-
                # otherwise-unread tiles for the pool allocator so their
                # liveness does not wrap around the loop back edge.
                h = body(deferred=True, prev=None)
                for _ in range(nbodies - 1):
                    h = body(deferred=True, prev=h)
                dumt = smp.tile([1, 8], f32, tag="dumt", name="dumt")
                nc.vector.tensor_copy(out=dumt[0:1, 0:1],
                                      in_=h["ov2"][1][0:1, 0:1])
                nc.vector.tensor_copy(out=dumt[0:1, 1:2],
                                      in_=h["otn"][0][0:1, 0:1])
                return h

            if repeat > 1:
                # unroll K bodies per hardware-loop iteration: the For_i
                # all-engine barrier + semaphore reset + ACT table reload
                # then amortize over K, and the K-1 internal iteration
                # boundaries pipeline freely through the Tile scheduler.
                K = 1
                n_blocks = repeat // K
                rem = repeat - n_blocks * K
                assert rem == 0, f"repeat={repeat} must be divisible by {K}"  # 2049 = 3*683
                # all but the last block run in the hardware loop; the last
                # block runs straight-line after it so its tiles can be read
                # by the tail without extending any lifetime across the
                # loop back edge. Its first body picks up the loop's final
                # pending state through the ring-address aliasing (the
                # For_i exit barrier orders the handoff).
                if n_blocks > 1:
                    with tc.For_i(0, n_blocks - 1, 1):
                        block(K)
                h = body(deferred=True, prev=None)
                for _ in range(K - 1):
                    h = body(deferred=True, prev=h)
                emit_tail(h)
            else:
                h = body(deferred=False)

    _split_multi_waits(nc)
    return nc


def _prep_host(inputs):
    """Host-side layout prep. Returns per-core input maps."""
    X = np.asarray(inputs["X"], dtype=np.float32)
    A = np.asarray(inputs["A"], dtype=np.float32)
    Wq = np.asarray(inputs["Wq"], dtype=np.float32)
    bq = np.asarray(inputs["bq"], dtype=np.float32)
    Wk = np.asarray(inputs["Wk"], dtype=np.float32)
    bk = np.asarray(inputs["bk"], dtype=np.float32)
    Wv = np.asarray(inputs["Wv"], dtype=np.float32)
    bv = np.asarray(inputs["bv"], dtype=np.float32)
    Wo = np.asarray(inputs["Wo"], dtype=np.float32)
    bo = np.asarray(inputs["bo"], dtype=np.float32)

    # grouped/padded QK weights: wt[g, c, 32a+d] = W[(4g+a)*16+d, c], d<16
    def qk_prep(W, b):
        W4 = W.reshape(2, 4, D, C)  # [g, a, d, c]
        wt = np.zeros((2, C, 4, 32), dtype=np.float32)
        wt[:, :, :, :D] = W4.transpose(0, 3, 1, 2)
        b4 = b.reshape(2, 4, D)
        bt = np.zeros((2, 4, 32), dtype=np.float32)
        bt[:, :, :D] = b4
        return wt.reshape(2, C, 128), bt.reshape(2, 128)

    wqt, bq2 = qk_prep(Wq, bq)
    wkt, bk2 = qk_prep(Wk, bk)
    wvt = Wv.T  # [c, f]
    # wot[g, 32a+d, f] = Wo[f, (4g+a)*16+d], d<16; pad rows stay zero
    Wo4 = Wo.reshape(F, 2, 4, D)  # [f, g, a, d]
    wot = np.zeros((2, 4, 32, F), dtype=np.float32)
    wot[:, :, :D, :] = Wo4.transpose(1, 2, 3, 0)
    wot = wot.reshape(2, 128, F)
    bfin = (bo + Wo @ bv).reshape(F)

    # weight blob [128, 7, 128]: wq g0,g1 | wk g0,g1 | wv | wo g0,g1
    wblob = np.stack([wqt[0], wqt[1], wkt[0], wkt[1], wvt, wot[0], wot[1]],
                     axis=1).astype(BF16)
    # bias blob [128, 5]: bq g0,g1 | bk g0,g1 | bfin
    bblob = np.stack([bq2[0], bq2[1], bk2[0], bk2[1], bfin],
                     axis=1).astype(np.float32)

    XT = X.transpose(0, 2, 1).astype(BF16)  # [b, c, n]
    # 0/1 adjacency, transposed and k-blocked: at01[b, p, j, q] = A[b, q, j*128+p]
    AT01 = (A.transpose(0, 2, 1) > 0).astype(BF16)  # [b, k, q]
    AT01 = np.ascontiguousarray(
        AT01.reshape(B, NB, 128, N).transpose(0, 2, 1, 3))  # [b, p, j, q]

    in_maps = []
    for b in range(B):
        in_maps.append({
            "xt": np.ascontiguousarray(XT[b]),
            "at01": AT01[b],
            "wblob": wblob, "bblob": bblob,
        })
    return in_maps


def run(inputs, trace=False):
    """Returns (output [B,N,F] float32, BassKernelResults)."""
    global _CACHED
    from concourse import bass_utils

    if _CACHED is None:
        _CACHED = _build_nc()
    nc = _CACHED
    in_maps = _prep_host(inputs)
    res = bass_utils.run_bass_kernel_spmd(
        nc, in_maps, core_ids=list(range(B)), trace=trace)
    out = np.stack([np.asarray(r["yt"], dtype=np.float32).T for r in res.results])
    return out, res


def kernel(**inputs):
    out, _ = run(inputs, trace=False)
    return out


def bench_loop(inputs, R=513, reps=6):
    """Device-side For_i repeat: per-kernel time = (wall_R - wall_1)/(R-1)."""
    import time
    from concourse import bass_utils

    in_maps = _prep_host(inputs)

    def timed(nc, reps):
        ts = []
        for _ in range(reps):
            t0 = time.perf_counter()
            bass_utils.run_bass_kernel_spmd(nc, in_maps, core_ids=list(range(B)))
            ts.append(time.perf_counter() - t0)
        return ts

    nc1 = _build_nc(1)
    ncR = _build_nc(R)
    timed(nc1, 2)  # warm both compiles
    timed(ncR, 2)
    t1s, tRs = [], []
    for _ in range(reps):
        t1s.extend(timed(nc1, 1))
        tRs.extend(timed(ncR, 1))
    t1, tR = min(t1s), min(tRs)
    per = (tR - t1) / (R - 1)
    return per, {"t1s": t1s, "tRs": tRs}


# revision 32
# speedup vs baseline: 1.2405x; 1.2405x over previous
"""Multi-head graph attention kernel for Trainium2 (8 NeuronCores).

Problem: B=8, N=1024, F_IN=F_OUT=128, H=8, D_K=16, sparse 0/1 adjacency mask.
Sharding: data-parallel over B - core b processes batch element b.

Math (identical to reference up to fp rounding):
    Q = X@Wq.T + bq ; K = X@Wk.T + bk ; V0 = X@Wv.T
    S = Q_h @ K_h.T ;  P = exp(S/4) * A01   (multiplicative 0/1 mask after
    exp == masked_fill(-1e9)+softmax numerator; softmax is shift-invariant
    and |S/4| is small so no max-subtract is needed)
    out = (P @ V0) / rowsum(P) @ Wo.T + (bo + Wo@bv)   (bv folds into bias
    because attn rows sum to 1)

Device layout (everything transposed on host so the device never transposes):
    xt   [128c, 1024n]      = X.T                   (bf16)
    at01 [128p, 8j, 1024q]  = A.T blocked over k    (bf16, 0/1)
    Heads are processed in two groups g in {0,1} of 4 heads; head slot a in
    {0..3} lives at partition base 32a with rows 16..31 zero-padded so the
    D_K=16 contraction can use 4-way tile_position row tiling on the PE.
    Scores are computed transposed: ST[k, q], one PSUM tile per
    (g, j-block, head-slot) covering the full q=1024 so the exp runs 1024
    wide and the adjacency multiply is a single contiguous DVE op at 2x
    bf16 rate. P.T streams straight into the P@V matmul, with a ones
    column appended to V giving rowsum for free; rowsum reciprocals are
    broadcast across each 32-partition quadrant with STREAM_SHUFFLE.
"""

import sys

sys.path.insert(0, "/opt/trn_rl_repo")

import numpy as np
import ml_dtypes

BF16 = ml_dtypes.bfloat16

B, N, C, F, H, D = 8, 1024, 128, 128, 8, 16
NB = N // 128  # 8 k-blocks

_CACHED = None
K_UNROLL = 1


def _split_multi_waits(nc):
    """This toolchain's walrus accepts at most ONE sync wait per instruction.
    Tile emits several; split the extras onto preceding same-engine NOPs."""
    import concourse.mybir as mybir

    for f in nc.m.functions:
        for blk in f.blocks:
            new = []
            for inst in blk.instructions:
                si = inst.sync_info
                if si is not None and si.on_wait is not None and len(si.on_wait) > 1:
                    waits = list(si.on_wait)
                    for w in waits[:-1]:
                        nop = mybir.InstNoOp(
                            name=nc.get_next_instruction_name(), ins=[], outs=[])
                        nop.engine = inst.engine
                        nop.sync_info = mybir.SyncInfo(on_wait=[w], on_update=[])
                        new.append(nop)
                    inst.sync_info = mybir.SyncInfo(
                        on_wait=[waits[-1]], on_update=list(si.on_update or []))
                new.append(inst)
            del blk.instructions[:]
            for i in new:
                blk.instructions.append(i)


def _build_nc(repeat=1):
    import concourse.bass as bass
    import concourse.tile as tile
    from concourse import mybir

    f32 = mybir.dt.float32
    bf16 = mybir.dt.bfloat16
    AF = mybir.ActivationFunctionType

    nc = bass.Bass()

    xt_d = nc.declare_dram_parameter("xt", [C, N], bf16, isOutput=False)
    at_d = nc.declare_dram_parameter("at01", [128, NB, N], bf16, isOutput=False)
    wblob_d = nc.declare_dram_parameter("wblob", [128, 7, 128], bf16, isOutput=False)
    bblob_d = nc.declare_dram_parameter("bblob", [128, 5], f32, isOutput=False)
    yt_d = nc.declare_dram_parameter("yt", [F, N], f32, isOutput=True)

    with tile.TileContext(nc) as tc:
        with (
            tc.tile_pool(name="consts", bufs=1) as cp,
            tc.tile_pool(name="sbwork", bufs=2) as wp,
            tc.tile_pool(name="ptp", bufs=10) as ptp,
            tc.tile_pool(name="smalls", bufs=2) as smp,
            tc.tile_pool(name="ps_big", bufs=2, space="PSUM") as pbig,
            tc.tile_pool(name="ps_ovy", bufs=1, space="PSUM") as povy,
            tc.tile_pool(name="dramp", bufs=2, space="DRAM") as dp,
        ):
            # ---- consts + input DMAs (outside the repeat loop) ----
            scratch = cp.tile([1, 8], f32, name="scratch")
            nc.vector.memset(scratch[:], 0.0)
            scratch2 = cp.tile([1, 8], f32, name="scratch2")
            # dummy exp: pulls the ACT table load to t=0 so it overlaps DMAs
            nc.scalar.activation(out=scratch2[:], in_=scratch[:], func=AF.Exp)

            wblob = cp.tile([128, 7, 128], bf16, name="wblob")
            nc.sync.dma_start(out=wblob[:], in_=wblob_d[:, :, :])
            xt_sb = cp.tile([C, N], bf16, name="xt_sb")
            nc.sync.dma_start(out=xt_sb[:], in_=xt_d[:, :])
            bblob = cp.tile([128, 5], f32, name="bblob")
            nc.sync.dma_start(out=bblob[:], in_=bblob_d[:, :])
            at_sb = cp.tile([128, NB, N], bf16, name="at_sb")
            nc.sync.dma_start(out=at_sb[:, 0:2, :], in_=at_d[:, 0:2, :])
            nc.sync.dma_start(out=at_sb[:, 2:NB, :], in_=at_d[:, 2:NB, :])

            wq_sb = [wblob[:, g, :] for g in range(2)]
            wk_sb = [wblob[:, 2 + g, :] for g in range(2)]
            wv_sb = wblob[:, 4, :]
            wo_sb = [wblob[:, 5 + g, :] for g in range(2)]
            bq_sb = [bblob[:, g : g + 1] for g in range(2)]
            bk_sb = [bblob[:, 2 + g : 3 + g] for g in range(2)]
            bfin_sb = bblob[:, 4:5]

            def body(deferred, prev=None):
                """One iteration. With deferred=True, the previous
                iteration's g1-normalization and output projection are
                finished here, interleaved into this iteration's g0
                attention. `prev` carries the previous body's accumulator
                handles when both sit in the same basic block (unrolled);
                across the hardware-loop back edge prev is None and the
                same work reads this emission's own tiles, whose addresses
                the ring reuse makes equal to the previous iteration's.
                emit_tail() finishes the last iteration."""
                qt_sb = [wp.tile([128, N], bf16, tag=f"qt{g}", name=f"qt{g}")
                         for g in range(2)]
                kt_sb = [wp.tile([128, N], bf16, tag=f"kt{g}", name=f"kt{g}")
                         for g in range(2)]
                vaug = wp.tile([128, NB, H, D + 1], bf16, tag="vaug",
                               name="vaug")
                otn = [wp.tile([128, N], bf16, tag=f"otn{g}", name=f"otn{g}")
                       for g in range(2)]
                yt_sb = wp.tile([F, N], f32, tag="yt_sb", name="yt_sb")
                ov2 = [None, None]
                if deferred and prev is None:
                    # first body of a block: allocate up front; the deferred
                    # reads below then target this emission's tiles, whose
                    # ring addresses alias the previous block's last body
                    ov2[0] = povy.tile([128, N], f32, tag="ovy0", name="ov0")
                    ov2[1] = povy.tile([128, N], f32, tag="ovy1", name="ov1")
                    prev = {"ov2": ov2, "otn": otn, "yt": yt_sb}

                def q_chunk(g):
                    qps = pbig.tile([128, N], f32, tag="big", name="qps")
                    nc.tensor.matmul(qps[:, 0:512], lhsT=wq_sb[g],
                                     rhs=xt_sb[:, 0:512])
                    nc.tensor.matmul(qps[:, 512:N], lhsT=wq_sb[g],
                                     rhs=xt_sb[:, 512:N])
                    nc.vector.tensor_scalar_add(qt_sb[g][:], qps[:], bq_sb[g])

                def k_chunk(g):
                    kps = pbig.tile([128, N], f32, tag="big", name="kps")
                    nc.tensor.matmul(kps[:, 0:512], lhsT=wk_sb[g],
                                     rhs=xt_sb[:, 0:512])
                    nc.tensor.matmul(kps[:, 512:N], lhsT=wk_sb[g],
                                     rhs=xt_sb[:, 512:N])
                    nc.vector.tensor_scalar_add(kt_sb[g][:], kps[:], bk_sb[g])

                def qk_chunk(g):
                    q_chunk(g)
                    k_chunk(g)

                vt_box = {}

                def vproj_chunk(half):
                    if half == 0:
                        vt_box["vt"] = pbig.tile([128, N], f32, tag="big",
                                                 name="vps")
                    vt = vt_box["vt"]
                    for j in range(4 * half, 4 * half + 4):
                        nc.tensor.matmul(vt[:, j * 128 : (j + 1) * 128],
                                         lhsT=xt_sb[:, j * 128 : (j + 1) * 128],
                                         rhs=wv_sb)

                def vcopy_chunk(half):
                    if half == 0:
                        nc.vector.memset(vaug[:, :, :, D : D + 1], 1.0)
                    vt = vt_box["vt"]
                    for j in range(4 * half, 4 * half + 4):
                        nc.vector.tensor_copy(
                            out=vaug[:, j, :, 0:D],
                            in_=vt[:, j * 128 : (j + 1) * 128].rearrange(
                                "p (h d) -> p h d", d=D))

                def norm_stages(g, n_ov2=None, n_otn=None):
                    # rowsums sit at partition 32a+16 of ov2[g]. Compact the
                    # 4 scattered rows to [128,32] via a DRAM round-trip (DMA
                    # handles strided/broadcast APs engines cannot), run the
                    # exact DVE reciprocal on 32 columns, broadcast each
                    # slot's reciprocal row over its 16 head partitions on
                    # the way back. Staged so the DMA latencies overlap the
                    # surrounding attention work.
                    ovs = smp.tile([128, N], f32, tag="ovs", name="ovs")
                    rsc = smp.tile([128, 32], f32, tag="rsc", name="rsc")
                    rsv = smp.tile([128, 32], f32, tag="rsv", name="rsv")
                    rsq = smp.tile([128, N], f32, tag="rsq", name="rsq")
                    rsd = dp.tile([4, N], f32, tag="rsd", name="rsd")
                    rsd2 = dp.tile([4, N], f32, tag="rsd2", name="rsd2")
                    if n_ov2 is None:
                        n_ov2, n_otn = ov2[g], otn[g]

                    def s1():
                        nc.vector.tensor_copy(out=ovs[:], in_=n_ov2[:])
                        for a in range(4):
                            nc.sync.dma_start(
                                out=rsd[a : a + 1, :],
                                in_=ovs[32 * a + D : 32 * a + D + 1, :])
                        d0 = rsd[0:1, 0:1]
                        nc.sync.dma_start(
                            out=rsc[:],
                            in_=bass.AP(tensor=d0.tensor, offset=d0.offset,
                                        ap=[[32, 128], [1, 32]]))

                    def s2():
                        nc.vector.reciprocal(out=rsv[:], in_=rsc[:])
                        d2 = rsd2[0:1, 0:1]
                        nc.sync.dma_start(
                            out=bass.AP(tensor=d2.tensor, offset=d2.offset,
                                        ap=[[32, 128], [1, 32]]),
                            in_=rsv[:])
                        for a in range(4):
                            # broadcast over the full 32-row quadrant (the
                            # 16 pad rows too) so no stale SBUF can leak
                            # inf/NaN into otn's pad rows
                            row = rsd2[a : a + 1, :]
                            nc.sync.dma_start(
                                out=rsq[32 * a : 32 * a + 32, :],
                                in_=bass.AP(tensor=row.tensor,
                                            offset=row.offset,
                                            ap=[[0, 32], [1, N]]))

                    def s3():
                        nc.vector.tensor_mul(n_otn[:], n_ov2[:], rsq[:])

                    return [s1, s2, s3]

                def phase3_chunk(qh, p_otn=None, p_yt=None):
                    if p_otn is None:
                        p_otn, p_yt = otn, yt_sb
                    q0 = qh * 512
                    yps = pbig.tile([128, N], f32, tag="big", name="yps")
                    nc.tensor.matmul(yps[:, 0:512], lhsT=wo_sb[0],
                                     rhs=p_otn[0][:, q0 : q0 + 512],
                                     start=True, stop=False)
                    nc.tensor.matmul(yps[:, 0:512], lhsT=wo_sb[1],
                                     rhs=p_otn[1][:, q0 : q0 + 512],
                                     start=False, stop=True)
                    nc.vector.tensor_scalar_add(p_yt[:, q0 : q0 + 512],
                                                yps[:, 0:512], bfin_sb)
                    nc.sync.dma_start(out=yt_d[:, q0 : q0 + 512],
                                      in_=p_yt[:, q0 : q0 + 512])

                def score_mm(g, j, a, qh, sps):
                    nc.tensor.matmul(
                        sps[:, qh * 512 : (qh + 1) * 512],
                        lhsT=kt_sb[g][32 * a : 32 * a + 32,
                                      j * 128 : (j + 1) * 128],
                        rhs=qt_sb[g][32 * a : 32 * a + 32,
                                     qh * 512 : (qh + 1) * 512],
                        start=True, stop=True,
                        skip_group_check=True,
                        tile_position=(32 * a, 0),
                    )

                def exp_mask(j, sps):
                    pt = ptp.tile([128, N], bf16, tag="pt", name="pt")
                    nc.scalar.activation(out=pt[:], in_=sps[:],
                                         func=AF.Exp, scale=0.25)
                    nc.vector.tensor_mul(pt[:], pt[:], at_sb[:, j, :])
                    return pt

                def pv_block(g, j, pts):
                    # qh-major: consecutive matmuls hit different 32-col
                    # strips of the array and stream concurrently
                    for qh in range(2):
                        for a in range(4):
                            h = 4 * g + a
                            nc.tensor.matmul(
                                ov2[g][32 * a : 32 * a + D + 1,
                                       qh * 512 : (qh + 1) * 512],
                                lhsT=vaug[:, j, h, :],
                                rhs=pts[a][:, qh * 512 : (qh + 1) * 512],
                                start=(j == 0),
                                stop=(j == NB - 1),
                                skip_group_check=True,
                                tile_position=(0, 32 * a),
                            )

                # body emission: Q/K of g0 first so the first scores (and
                # the ACT pipeline) restart with minimal latency at the
                # iteration boundary; everything else trickles in as
                # pending chunks popped between attention steps.
                qk_chunk(0)
                pending = [lambda: vproj_chunk(0), lambda: vcopy_chunk(0),
                           lambda: vproj_chunk(1), lambda: vcopy_chunk(1),
                           lambda: q_chunk(1), lambda: k_chunk(1)]
                if deferred:
                    # finish the previous body: its g1 normalization and
                    # output projection, interleaved into our g0 attention
                    pending += norm_stages(1, prev["ov2"][1],
                                           prev["otn"][1])
                    pending += [lambda: phase3_chunk(0, prev["otn"],
                                                     prev["yt"]),
                                lambda: phase3_chunk(1, prev["otn"],
                                                     prev["yt"])]

                pvprev = None
                for g in range(2):
                    if ov2[g] is None:
                        ov2[g] = povy.tile([128, N], f32, tag=f"ovy{g}",
                                           name=f"ov{g}")
                    for j in range(NB):
                        pair01 = [pbig.tile([128, N], f32, tag="big",
                                            name="sps") for _ in range(2)]
                        for qh in range(2):
                            for si in range(2):
                                score_mm(g, j, si, qh, pair01[si])
                        if pvprev is not None:
                            pv_block(*pvprev)
                        if pending:
                            pending.pop(0)()
                        if j == 0 and pending:
                            # vcopy must land before this j's PV block
                            pending.pop(0)()
                        pair23 = [pbig.tile([128, N], f32, tag="big",
                                            name="sps") for _ in range(2)]
                        for qh in range(2):
                            for si in range(2):
                                score_mm(g, j, 2 + si, qh, pair23[si])
                        pts = [exp_mask(j, s) for s in pair01 + pair23]
                        pvprev = (g, j, pts)
                    pv_block(*pvprev)
                    pvprev = None
                    if g == 0:
                        pending += norm_stages(0)
                for p in pending:
                    p()

                if not deferred:
                    for s in norm_stages(1):
                        s()
                    phase3_chunk(0)
                    phase3_chunk(1)
                return {"ov2": ov2, "otn": otn, "yt": yt_sb,
                        "norm_stages": norm_stages,
                        "phase3_chunk": phase3_chunk}

            def emit_tail(h):
                for s in h["norm_stages"](1):
                    s()
                h["phase3_chunk"](0)
                h["phase3_chunk"](1)

            def block(nbodies):
                # group of iterations: body k+1 finishes body k's tail work
                # interleaved into its own attention; the LAST body's
                # accumulators are finished by the NEXT block's first body
                # through ring-address aliasing (the For_i barrier orders
                # it). The tiny reads below retire those written-but-
                # otherwise-unread tiles for the pool allocator so their
                # liveness does not wrap around the loop back edge.
                h = body(deferred=True, prev=None)
                for _ in range(nbodies - 1):
                    h = body(deferred=True, prev=h)
                dumt = smp.tile([1, 8], f32, tag="dumt", name="dumt")
                nc.vector.tensor_copy(out=dumt[0:1, 0:1],
                                      in_=h["ov2"][1][0:1, 0:1])
                nc.vector.tensor_copy(out=dumt[0:1, 1:2],
                                      in_=h["otn"][0][0:1, 0:1])
                return h

            if repeat > 1:
                # unroll K bodies per hardware-loop iteration: the For_i
                # all-engine barrier + semaphore reset + ACT table reload
                # then amortize over K, and the K-1 internal iteration
                # boundaries pipeline freely through the Tile scheduler.
                K = K_UNROLL
                n_blocks = repeat // K
                rem = repeat - n_blocks * K
                assert rem == 0, f"repeat={repeat} must be divisible by {K}"  # 2049 = 3*683
                # all but the last block run in the hardware loop; the last
                # block runs straight-line after it so its tiles can be read
                # by the tail without extending any lifetime across the
                # loop back edge. Its first body picks up the loop's final
                # pending state through the ring-address aliasing (the
                # For_i exit barrier orders the handoff).
                if n_blocks > 1:
                    with tc.For_i(0, n_blocks - 1, 1):
                        block(K)
                h = body(deferred=True, prev=None)
                for _ in range(K - 1):
                    h = body(deferred=True, prev=h)
                emit_tail(h)
            else:
                h = body(deferred=False)

    _split_multi_waits(nc)
    return nc


def _prep_host(inputs):
    """Host-side layout prep. Returns per-core input maps."""
    X = np.asarray(inputs["X"], dtype=np.float32)
    A = np.asarray(inputs["A"], dtype=np.float32)
    Wq = np.asarray(inputs["Wq"], dtype=np.float32)
    bq = np.asarray(inputs["bq"], dtype=np.float32)
    Wk = np.asarray(inputs["Wk"], dtype=np.float32)
    bk = np.asarray(inputs["bk"], dtype=np.float32)
    Wv = np.asarray(inputs["Wv"], dtype=np.float32)
    bv = np.asarray(inputs["bv"], dtype=np.float32)
    Wo = np.asarray(inputs["Wo"], dtype=np.float32)
    bo = np.asarray(inputs["bo"], dtype=np.float32)

    # grouped/padded QK weights: wt[g, c, 32a+d] = W[(4g+a)*16+d, c], d<16
    def qk_prep(W, b):
        W4 = W.reshape(2, 4, D, C)  # [g, a, d, c]
        wt = np.zeros((2, C, 4, 32), dtype=np.float32)
        wt[:, :, :, :D] = W4.transpose(0, 3, 1, 2)
        b4 = b.reshape(2, 4, D)
        bt = np.zeros((2, 4, 32), dtype=np.float32)
        bt[:, :, :D] = b4
        return wt.reshape(2, C, 128), bt.reshape(2, 128)

    wqt, bq2 = qk_prep(Wq, bq)
    wkt, bk2 = qk_prep(Wk, bk)
    wvt = Wv.T  # [c, f]
    # wot[g, 32a+d, f] = Wo[f, (4g+a)*16+d], d<16; pad rows stay zero
    Wo4 = Wo.reshape(F, 2, 4, D)  # [f, g, a, d]
    wot = np.zeros((2, 4, 32, F), dtype=np.float32)
    wot[:, :, :D, :] = Wo4.transpose(1, 2, 3, 0)
    wot = wot.reshape(2, 128, F)
    bfin = (bo + Wo @ bv).reshape(F)

    # weight blob [128, 7, 128]: wq g0,g1 | wk g0,g1 | wv | wo g0,g1
    wblob = np.stack([wqt[0], wqt[1], wkt[0], wkt[1], wvt, wot[0], wot[1]],
                     axis=1).astype(BF16)
    # bias blob [128, 5]: bq g0,g1 | bk g0,g1 | bfin
    bblob = np.stack([bq2[0], bq2[1], bk2[0], bk2[1], bfin],
                     axis=1).astype(np.float32)

    XT = X.transpose(0, 2, 1).astype(BF16)  # [b, c, n]
    # 0/1 adjacency, transposed and k-blocked: at01[b, p, j, q] = A[b, q, j*128+p]
    AT01 = (A.transpose(0, 2, 1) > 0).astype(BF16)  # [b, k, q]
    AT01 = np.ascontiguousarray(
        AT01.reshape(B, NB, 128, N).transpose(0, 2, 1, 3))  # [b, p, j, q]

    in_maps = []
    for b in range(B):
        in_maps.append({
            "xt": np.ascontiguousarray(XT[b]),
            "at01": AT01[b],
            "wblob": wblob, "bblob": bblob,
        })
    return in_maps


def run(inputs, trace=False):
    """Returns (output [B,N,F] float32, BassKernelResults)."""
    global _CACHED
    from concourse import bass_utils

    if _CACHED is None:
        _CACHED = _build_nc()
    nc = _CACHED
    in_maps = _prep_host(inputs)
    res = bass_utils.run_bass_kernel_spmd(
        nc, in_maps, core_ids=list(range(B)), trace=trace)
    out = np.stack([np.asarray(r["yt"], dtype=np.float32).T for r in res.results])
    return out, res


def kernel(**inputs):
    out, _ = run(inputs, trace=False)
    return out


def bench_loop(inputs, R=513, reps=6):
    """Device-side For_i repeat: per-kernel time = (wall_R - wall_1)/(R-1)."""
    import time
    from concourse import bass_utils

    in_maps = _prep_host(inputs)

    def timed(nc, reps):
        ts = []
        for _ in range(reps):
            t0 = time.perf_counter()
            bass_utils.run_bass_kernel_spmd(nc, in_maps, core_ids=list(range(B)))
            ts.append(time.perf_counter() - t0)
        return ts

    nc1 = _build_nc(1)
    ncR = _build_nc(R)
    timed(nc1, 2)  # warm both compiles
    timed(ncR, 2)
    t1s, tRs = [], []
    for _ in range(reps):
        t1s.extend(timed(nc1, 1))
        tRs.extend(timed(ncR, 1))
    t1, tR = min(t1s), min(tRs)
    per = (tR - t1) / (R - 1)
    return per, {"t1s": t1s, "tRs": tRs}
